# revision 2
# baseline (speedup 1.0000x reference)
"""Trainium2 Bass kernel for nn_Nodes_Embedding (dense MLP + per-atom scalar add).

out[i] = ELU3(x[i]) + atom_part[i] + atom_charge[atom_part[i]]

Strategy (pure data parallel over 8 cores):
  - Host: gather add[i] = atom_part[i] + atom_charge[atom_part[i]] - 1 (tiny 1024-entry
    table, O(N) numpy), cast x to bf16, shard atoms across 8 cores.
  - Device (per core, feature-major layout):
      * x loaded via DMA-xbar-transpose into [128, C] bf16 tiles: partition 16g+f holds
        feature f of interleaved atom group g (atom = base + 8*t + g).
      * 3 MLP layers as block-diagonal [128,128] bf16 matmuls (8 copies of Wl.T).
      * ELU(x) = max(x,0) + min(exp(x),1) - 1:
          ACT: t = Exp(psum + b)   (free per-partition bias)
          DVE: r = max(psum + b, 0)  (dual-op tensor_scalar)
          DVE/GPSIMD: m = min(t, 1)
        r and m fed as two accumulated matmuls into the next layer ("-1" folded into
        the next layer's bias: b' = b - W @ ones).
      * transpose back via PE (bf16), final DVE pass adds the fp32 per-atom scalar
        (broadcast along features) and writes fp32 output tiles, contiguous DMA out.
"""

import os
import sys
import functools

import numpy as np

sys.path.insert(0, "/opt/trn_rl_repo")

import ml_dtypes

BF16 = ml_dtypes.bfloat16

D = 16
G = 8                       # partition groups of 16 features
MEGA_COLS = 2048            # columns per load megatile (per group)
MEGA_ATOMS = G * MEGA_COLS  # 16384 atoms per megatile
CHUNK_COLS = 1024           # compute chunk (2-bank PSUM)
MM_COLS = 512               # single matmul free dim
NCORES = 8

N_ATOMS = 4_000_000
N_PARTS = 1024


def _round_up(a, b):
    return (a + b - 1) // b * b


@functools.lru_cache(maxsize=4)
def _build_program(megas, min_on_gpsimd=True):
    """Build the per-core Bass program for `megas` megatiles of atoms."""
    import concourse.bass as bass
    import concourse.bacc as bacc
    import concourse.tile as tile
    from concourse import mybir
    from contextlib import ExitStack

    dt = mybir.dt
    AF = mybir.ActivationFunctionType
    OP = mybir.AluOpType

    npc = megas * MEGA_ATOMS

    nc = bacc.Bacc("TRN2", target_bir_lowering=False, debug=False)

    x_d = nc.dram_tensor("x", [npc, D], dt.bfloat16, kind="ExternalInput").ap()
    addT_d = nc.dram_tensor(
        "addT", [128, npc // 128], dt.float32, kind="ExternalInput"
    ).ap()
    lt_d = [
        nc.dram_tensor(f"lhsT{l}", [128, 128], dt.bfloat16, kind="ExternalInput").ap()
        for l in (1, 2, 3)
    ]
    bias_d = [
        nc.dram_tensor(f"bias{l}", [128, 1], dt.float32, kind="ExternalInput").ap()
        for l in (1, 2, 3)
    ]
    ident_d = nc.dram_tensor(
        "ident", [128, 128], dt.bfloat16, kind="ExternalInput"
    ).ap()
    out_d = nc.dram_tensor("out", [npc, D], dt.float32, kind="ExternalOutput").ap()

    # atom = m*MEGA_ATOMS + 8*t + e  (interleaved groups)
    x3 = x_d.rearrange("(m t e) d -> m e t d", e=G, t=MEGA_COLS)
    # output blocks of 1024 atoms: atom = q*1024 + 8*t + e
    out5 = out_d.rearrange("(q t e) d -> t q (e d)", t=128, e=G)

    with ExitStack() as ctx:
        tc = ctx.enter_context(tile.TileContext(nc))
        consts = ctx.enter_context(tc.tile_pool(name="consts", bufs=1))
        xpool = ctx.enter_context(tc.tile_pool(name="xpool", bufs=2))
        actp = ctx.enter_context(tc.tile_pool(name="actp", bufs=2))
        addp = ctx.enter_context(tc.tile_pool(name="addp", bufs=2))
        outp = ctx.enter_context(tc.tile_pool(name="outp", bufs=3))
        psums = ctx.enter_context(tc.tile_pool(name="psums", bufs=3, space="PSUM"))
        psumT = ctx.enter_context(tc.tile_pool(name="psumT", bufs=2, space="PSUM"))

        LT = []
        for l in range(3):
            t = consts.tile([128, 128], dt.bfloat16, tag=f"lt{l}")
            nc.sync.dma_start(t, lt_d[l])
            LT.append(t)
        BIAS = []
        for l in range(3):
            t = consts.tile([128, 1], dt.float32, tag=f"b{l}")
            nc.sync.dma_start(t, bias_d[l])
            BIAS.append(t)
        IDENT = consts.tile([128, 128], dt.bfloat16, tag="ident")
        nc.sync.dma_start(IDENT, ident_d)

        n_chunks = MEGA_COLS // CHUNK_COLS
        halves = CHUNK_COLS // MM_COLS

        for m in range(megas):
            X = xpool.tile([128, MEGA_COLS], dt.bfloat16, tag="X")
            for g in range(G):
                nc.sync.dma_start_transpose(X[16 * g : 16 * (g + 1), :], x3[m, g])
            # add values for this megatile: [128, 4*8] per 4096-atom block, 4 blocks
            AT = addp.tile([128, MEGA_ATOMS // 128], dt.float32, tag="AT")
            nc.sync.dma_start(
                AT, addT_d[:, m * (MEGA_ATOMS // 128) : (m + 1) * (MEGA_ATOMS // 128)]
            )

            for c in range(n_chunks):
                Xc = X[:, c * CHUNK_COLS : (c + 1) * CHUNK_COLS]

                # ---- layer 1
                ps1 = psums.tile([128, CHUNK_COLS], dt.float32, tag="ps")
                for h in range(halves):
                    s = slice(h * MM_COLS, (h + 1) * MM_COLS)
                    nc.tensor.matmul(ps1[:, s], LT[0], Xc[:, s], start=True, stop=True)
                T1 = actp.tile([128, CHUNK_COLS], dt.bfloat16, tag="T")
                nc.scalar.activation(T1, ps1, AF.Exp, bias=BIAS[0])
                R1 = actp.tile([128, CHUNK_COLS], dt.bfloat16, tag="R")
                nc.vector.tensor_scalar(
                    out=R1, in0=ps1, scalar1=BIAS[0], scalar2=0.0,
                    op0=OP.add, op1=OP.max,
                )
                M1 = actp.tile([128, CHUNK_COLS], dt.bfloat16, tag="M")
                eng1 = nc.gpsimd if min_on_gpsimd else nc.vector
                eng1.tensor_scalar(
                    out=M1, in0=T1, scalar1=1.0, scalar2=None, op0=OP.min
                )

                # ---- layer 2
                ps2 = psums.tile([128, CHUNK_COLS], dt.float32, tag="ps")
                for h in range(halves):
                    s = slice(h * MM_COLS, (h + 1) * MM_COLS)
                    nc.tensor.matmul(ps2[:, s], LT[1], R1[:, s], start=True, stop=False)
                    nc.tensor.matmul(ps2[:, s], LT[1], M1[:, s], start=False, stop=True)
                T2 = actp.tile([128, CHUNK_COLS], dt.bfloat16, tag="T")
                nc.scalar.activation(T2, ps2, AF.Exp, bias=BIAS[1])
                R2 = actp.tile([128, CHUNK_COLS], dt.bfloat16, tag="R")
                nc.vector.tensor_scalar(
                    out=R2, in0=ps2, scalar1=BIAS[1], scalar2=0.0,
                    op0=OP.add, op1=OP.max,
                )
                M2 = actp.tile([128, CHUNK_COLS], dt.bfloat16, tag="M")
                eng1.tensor_scalar(
                    out=M2, in0=T2, scalar1=1.0, scalar2=None, op0=OP.min
                )

                # ---- layer 3
                ps3 = psums.tile([128, CHUNK_COLS], dt.float32, tag="ps")
                for h in range(halves):
                    s = slice(h * MM_COLS, (h + 1) * MM_COLS)
                    nc.tensor.matmul(ps3[:, s], LT[2], R2[:, s], start=True, stop=False)
                    nc.tensor.matmul(ps3[:, s], LT[2], M2[:, s], start=False, stop=True)
                T3 = actp.tile([128, CHUNK_COLS], dt.bfloat16, tag="T")
                nc.scalar.activation(T3, ps3, AF.Exp, bias=BIAS[2])
                R3 = actp.tile([128, CHUNK_COLS], dt.bfloat16, tag="R")
                nc.vector.tensor_scalar(
                    out=R3, in0=ps3, scalar1=BIAS[2], scalar2=0.0,
                    op0=OP.add, op1=OP.max,
                )
                M3 = actp.tile([128, CHUNK_COLS], dt.bfloat16, tag="M")
                eng1.tensor_scalar(
                    out=M3, in0=T3, scalar1=1.0, scalar2=None, op0=OP.min
                )
                C = actp.tile([128, CHUNK_COLS], dt.bfloat16, tag="C")
                nc.vector.tensor_tensor(out=C, in0=R3, in1=M3, op=OP.add)

                # ---- transpose back + final add, per 512 cols (4096 atoms)
                for half in range(halves):
                    pT = psumT.tile([128, MM_COLS], dt.bfloat16, tag="pT")
                    for k in range(4):
                        nc.tensor.transpose(
                            pT[:, k * 128 : (k + 1) * 128],
                            C[:, half * MM_COLS + k * 128 : half * MM_COLS + (k + 1) * 128],
                            IDENT,
                        )
                    osb = outp.tile([128, MM_COLS], dt.float32, tag="osb")
                    # add slice for this 4096-atom span: [128, 4 blocks, 8 groups]
                    a0 = (c * halves + half) * 32
                    sl = AT[:, a0 : a0 + 32]
                    add_b = bass.AP(
                        tensor=sl.tensor,
                        offset=sl.offset,
                        ap=[sl.ap[0], [8, 4], [1, 8], [0, 16]],
                    )
                    nc.vector.tensor_tensor(
                        out=osb.rearrange("p (k g f) -> p k g f", k=4, g=8),
                        in0=pT.rearrange("p (k g f) -> p k g f", k=4, g=8),
                        in1=add_b,
                        op=OP.add,
                    )
                    q0 = m * (MEGA_ATOMS // 1024) + (c * halves + half) * 4
                    nc.sync.dma_start(
                        out5[:, q0 : q0 + 4, :],
                        osb.rearrange("p (k f) -> p k f", k=4),
                    )

    nc.compile()
    return nc


def _host_prep(atom_type_vector, atom_charge, W1, b1, W2, b2, W3, b3, atom_part):
    """Host-side: gather table, pad, shard, reorder. Returns (in_maps, n, npc)."""
    x = np.asarray(atom_type_vector, dtype=np.float32)
    charge = np.asarray(atom_charge, dtype=np.float32)
    part = np.asarray(atom_part).astype(np.int64)
    W1 = np.asarray(W1, np.float32); b1 = np.asarray(b1, np.float32)
    W2 = np.asarray(W2, np.float32); b2 = np.asarray(b2, np.float32)
    W3 = np.asarray(W3, np.float32); b3 = np.asarray(b3, np.float32)

    n = x.shape[0]
    npc = _round_up(_round_up(n, NCORES) // NCORES, MEGA_ATOMS)
    ntot = npc * NCORES

    # table2[j] = j + charge[j] - 1   (the ELU "-1" of layer 3 folded in)
    table2 = np.arange(charge.shape[0], dtype=np.float32) + charge - 1.0
    add = table2[part]

    x_pad = np.zeros((ntot, D), dtype=BF16)
    x_pad[:n] = x.astype(BF16)
    add_pad = np.zeros(ntot, dtype=np.float32)
    add_pad[:n] = add

    # weights: block-diag lhsT, biases with the previous layer's "-1" folded
    lhsT = [np.kron(np.eye(G, dtype=np.float32), W.T).astype(BF16) for W in (W1, W2, W3)]
    b2p = b2 - W2.sum(axis=1)
    b3p = b3 - W3.sum(axis=1)
    biases = [np.tile(b, G).reshape(128, 1).astype(np.float32) for b in (b1, b2p, b3p)]
    ident = np.eye(128, dtype=np.float32).astype(BF16)

    in_maps = []
    for c in range(NCORES):
        xs = x_pad[c * npc : (c + 1) * npc]
        ad = add_pad[c * npc : (c + 1) * npc]
        # addT[t, b*32 + k*8 + g] = add[b*4096 + k*1024 + t*8 + g]
        addT = (
            ad.reshape(-1, 4, 128, G).transpose(2, 0, 1, 3).reshape(128, -1)
        ).copy()
        in_maps.append(
            {
                "x": xs,
                "addT": addT,
                "lhsT1": lhsT[0], "lhsT2": lhsT[1], "lhsT3": lhsT[2],
                "bias1": biases[0], "bias2": biases[1], "bias3": biases[2],
                "ident": ident,
            }
        )
    return in_maps, n, npc


def kernel(**inputs) -> np.ndarray:
    from concourse.bass_utils import run_bass_kernel_spmd

    in_maps, n, npc = _host_prep(**inputs)
    nc = _build_program(npc // MEGA_ATOMS)
    res = run_bass_kernel_spmd(nc, in_maps, core_ids=list(range(NCORES)))
    out = np.concatenate([res.results[c]["out"] for c in range(NCORES)], axis=0)
    return out[:n].astype(np.float32)


# ------------------------- numpy reference of the device math -------------------
def numpy_ref(x, charge, W1, b1, W2, b2, W3, b3, part):
    def elu(v):
        return np.where(v > 0, v, np.expm1(v))

    h = elu(x @ W1.T + b1)
    h = elu(h @ W2.T + b2)
    h = elu(h @ W3.T + b3)
    add = part.astype(np.float32) + charge[part]
    return h + add[:, None]


# revision 5
# speedup vs baseline: 25.2967x; 25.2967x over previous
"""Trainium2 Bass kernel for nn_Nodes_Embedding (dense MLP + per-atom scalar add).

out[i] = ELU3(x[i]) + atom_part[i] + atom_charge[atom_part[i]]

Strategy (pure data parallel over 8 cores):
  - Host: gather add[i] = atom_part[i] + atom_charge[atom_part[i]] - 1 (tiny 1024-entry
    table, O(N) numpy), cast x to bf16, shard atoms across 8 cores.
  - Device (per core, feature-major layout):
      * x loaded via DMA-xbar-transpose into [128, C] bf16 tiles: partition 16g+f holds
        feature f of interleaved atom group g (atom = base + 8*t + g).
      * 3 MLP layers as block-diagonal [128,128] bf16 matmuls (8 copies of Wl.T).
      * ELU(x) = max(x,0) + min(exp(x),1) - 1:
          ACT: t = Exp(psum + b)   (free per-partition bias)
          DVE: r = max(psum + b, 0)  (dual-op tensor_scalar)
          DVE/GPSIMD: m = min(t, 1)
        r and m fed as two accumulated matmuls into the next layer ("-1" folded into
        the next layer's bias: b' = b - W @ ones).
      * transpose back via PE (bf16), final DVE pass adds the fp32 per-atom scalar
        (broadcast along features) and writes fp32 output tiles, contiguous DMA out.
"""

import os
import sys
import functools

import numpy as np

sys.path.insert(0, "/opt/trn_rl_repo")

import ml_dtypes

BF16 = ml_dtypes.bfloat16

D = 16
G = 8                       # partition groups of 16 features
MEGA_COLS = 2048            # columns per load megatile (per group)
MEGA_ATOMS = G * MEGA_COLS  # 16384 atoms per megatile
CHUNK_COLS = 1024           # compute chunk (2-bank PSUM)
MM_COLS = 512               # single matmul free dim
NCORES = 8

N_ATOMS = 4_000_000
N_PARTS = 1024


def _round_up(a, b):
    return (a + b - 1) // b * b


@functools.lru_cache(maxsize=4)
def _build_program(megas, min_on_gpsimd=False):
    """Build the per-core Bass program for `megas` megatiles of atoms."""
    import concourse.bass as bass
    import concourse.bacc as bacc
    import concourse.tile as tile
    from concourse import mybir
    from contextlib import ExitStack

    dt = mybir.dt
    AF = mybir.ActivationFunctionType
    OP = mybir.AluOpType

    npc = megas * MEGA_ATOMS

    nc = bacc.Bacc("TRN2", target_bir_lowering=False, debug=False)

    x_d = nc.dram_tensor("x", [npc, D], dt.bfloat16, kind="ExternalInput").ap()
    addT_d = nc.dram_tensor(
        "addT", [128, npc // 128], dt.float32, kind="ExternalInput"
    ).ap()
    lt_d = [
        nc.dram_tensor(f"lhsT{l}", [128, 128], dt.bfloat16, kind="ExternalInput").ap()
        for l in (1, 2, 3)
    ]
    bias_d = [
        nc.dram_tensor(f"bias{l}", [128, 1], dt.float32, kind="ExternalInput").ap()
        for l in (1, 2, 3)
    ]
    ident_d = nc.dram_tensor(
        "ident", [128, 128], dt.bfloat16, kind="ExternalInput"
    ).ap()
    out_d = nc.dram_tensor("out", [npc, D], dt.float32, kind="ExternalOutput").ap()

    # atom = m*MEGA_ATOMS + 8*t + e  (interleaved groups)
    # One contiguous xbar transpose per megatile: source rows of 128 elements
    # (= 8 atoms x 16 feats), so dest partition 16g+f = feat f of atom 8t+g.
    x2 = x_d.rearrange("(m t e) d -> m t (e d)", e=G, t=MEGA_COLS)
    # output blocks of 1024 atoms: atom = q*1024 + 8*t + e
    out5 = out_d.rearrange("(q t e) d -> t q (e d)", t=128, e=G)

    with ExitStack() as ctx:
        tc = ctx.enter_context(tile.TileContext(nc))
        consts = ctx.enter_context(tc.tile_pool(name="consts", bufs=1))
        xpool = ctx.enter_context(tc.tile_pool(name="xpool", bufs=2))
        actp = ctx.enter_context(tc.tile_pool(name="actp", bufs=2))
        addp = ctx.enter_context(tc.tile_pool(name="addp", bufs=2))
        outp = ctx.enter_context(tc.tile_pool(name="outp", bufs=3))
        psums = ctx.enter_context(tc.tile_pool(name="psums", bufs=3, space="PSUM"))
        psumT = ctx.enter_context(tc.tile_pool(name="psumT", bufs=2, space="PSUM"))

        LT = []
        for l in range(3):
            t = consts.tile([128, 128], dt.bfloat16, tag=f"lt{l}")
            nc.sync.dma_start(t, lt_d[l])
            LT.append(t)
        BIAS = []
        for l in range(3):
            t = consts.tile([128, 1], dt.float32, tag=f"b{l}")
            nc.sync.dma_start(t, bias_d[l])
            BIAS.append(t)
        IDENT = consts.tile([128, 128], dt.bfloat16, tag="ident")
        nc.sync.dma_start(IDENT, ident_d)

        n_chunks = MEGA_COLS // CHUNK_COLS
        halves = CHUNK_COLS // MM_COLS

        for m in range(megas):
            X = xpool.tile([128, MEGA_COLS], dt.bfloat16, tag="X")
            nc.sync.dma_start_transpose(X, x2[m])
            # add values for this megatile: [128, 4*8] per 4096-atom block, 4 blocks
            AT = addp.tile([128, MEGA_ATOMS // 128], dt.float32, tag="AT")
            nc.sync.dma_start(
                AT, addT_d[:, m * (MEGA_ATOMS // 128) : (m + 1) * (MEGA_ATOMS // 128)]
            )

            for c in range(n_chunks):
                Xc = X[:, c * CHUNK_COLS : (c + 1) * CHUNK_COLS]

                # ---- layer 1
                ps1 = psums.tile([128, CHUNK_COLS], dt.float32, tag="ps")
                for h in range(halves):
                    s = slice(h * MM_COLS, (h + 1) * MM_COLS)
                    nc.tensor.matmul(ps1[:, s], LT[0], Xc[:, s], start=True, stop=True)
                T1 = actp.tile([128, CHUNK_COLS], dt.bfloat16, tag="T")
                nc.scalar.activation(T1, ps1, AF.Exp, bias=BIAS[0])
                R1 = actp.tile([128, CHUNK_COLS], dt.bfloat16, tag="R")
                nc.vector.tensor_scalar(
                    out=R1, in0=ps1, scalar1=BIAS[0], scalar2=0.0,
                    op0=OP.add, op1=OP.max,
                )
                M1 = actp.tile([128, CHUNK_COLS], dt.bfloat16, tag="M")
                eng1 = nc.gpsimd if min_on_gpsimd else nc.vector
                eng1.tensor_scalar(
                    out=M1, in0=T1, scalar1=1.0, scalar2=None, op0=OP.min
                )

                # ---- layer 2
                ps2 = psums.tile([128, CHUNK_COLS], dt.float32, tag="ps")
                for h in range(halves):
                    s = slice(h * MM_COLS, (h + 1) * MM_COLS)
                    nc.tensor.matmul(ps2[:, s], LT[1], R1[:, s], start=True, stop=False)
                    nc.tensor.matmul(ps2[:, s], LT[1], M1[:, s], start=False, stop=True)
                T2 = actp.tile([128, CHUNK_COLS], dt.bfloat16, tag="T")
                nc.scalar.activation(T2, ps2, AF.Exp, bias=BIAS[1])
                R2 = actp.tile([128, CHUNK_COLS], dt.bfloat16, tag="R")
                nc.vector.tensor_scalar(
                    out=R2, in0=ps2, scalar1=BIAS[1], scalar2=0.0,
                    op0=OP.add, op1=OP.max,
                )
                M2 = actp.tile([128, CHUNK_COLS], dt.bfloat16, tag="M")
                eng1.tensor_scalar(
                    out=M2, in0=T2, scalar1=1.0, scalar2=None, op0=OP.min
                )

                # ---- layer 3
                ps3 = psums.tile([128, CHUNK_COLS], dt.float32, tag="ps")
                for h in range(halves):
                    s = slice(h * MM_COLS, (h + 1) * MM_COLS)
                    nc.tensor.matmul(ps3[:, s], LT[2], R2[:, s], start=True, stop=False)
                    nc.tensor.matmul(ps3[:, s], LT[2], M2[:, s], start=False, stop=True)
                T3 = actp.tile([128, CHUNK_COLS], dt.bfloat16, tag="T")
                nc.scalar.activation(T3, ps3, AF.Exp, bias=BIAS[2])
                R3 = actp.tile([128, CHUNK_COLS], dt.bfloat16, tag="R")
                nc.vector.tensor_scalar(
                    out=R3, in0=ps3, scalar1=BIAS[2], scalar2=0.0,
                    op0=OP.add, op1=OP.max,
                )
                M3 = actp.tile([128, CHUNK_COLS], dt.bfloat16, tag="M")
                eng1.tensor_scalar(
                    out=M3, in0=T3, scalar1=1.0, scalar2=None, op0=OP.min
                )
                C = actp.tile([128, CHUNK_COLS], dt.bfloat16, tag="C")
                nc.vector.tensor_tensor(out=C, in0=R3, in1=M3, op=OP.add)

                # ---- transpose back + final add, per 512 cols (4096 atoms)
                for half in range(halves):
                    pT = psumT.tile([128, MM_COLS], dt.bfloat16, tag="pT")
                    for k in range(4):
                        nc.tensor.transpose(
                            pT[:, k * 128 : (k + 1) * 128],
                            C[:, half * MM_COLS + k * 128 : half * MM_COLS + (k + 1) * 128],
                            IDENT,
                        )
                    osb = outp.tile([128, MM_COLS], dt.float32, tag="osb")
                    # add slice for this 4096-atom span: [128, 4 blocks, 8 groups]
                    a0 = (c * halves + half) * 32
                    sl = AT[:, a0 : a0 + 32]
                    add_b = bass.AP(
                        tensor=sl.tensor,
                        offset=sl.offset,
                        ap=[sl.ap[0], [8, 4], [1, 8], [0, 16]],
                    )
                    nc.vector.tensor_tensor(
                        out=osb.rearrange("p (k g f) -> p k g f", k=4, g=8),
                        in0=pT.rearrange("p (k g f) -> p k g f", k=4, g=8),
                        in1=add_b,
                        op=OP.add,
                    )
                    q0 = m * (MEGA_ATOMS // 1024) + (c * halves + half) * 4
                    nc.sync.dma_start(
                        out5[:, q0 : q0 + 4, :],
                        osb.rearrange("p (k f) -> p k f", k=4),
                    )

    nc.compile()
    return nc


def _host_prep(atom_type_vector, atom_charge, W1, b1, W2, b2, W3, b3, atom_part):
    """Host-side: gather table, pad, shard, reorder. Returns (in_maps, n, npc)."""
    x = np.asarray(atom_type_vector, dtype=np.float32)
    charge = np.asarray(atom_charge, dtype=np.float32)
    part = np.asarray(atom_part).astype(np.int64)
    W1 = np.asarray(W1, np.float32); b1 = np.asarray(b1, np.float32)
    W2 = np.asarray(W2, np.float32); b2 = np.asarray(b2, np.float32)
    W3 = np.asarray(W3, np.float32); b3 = np.asarray(b3, np.float32)

    n = x.shape[0]
    npc = _round_up(_round_up(n, NCORES) // NCORES, MEGA_ATOMS)
    ntot = npc * NCORES

    # table2[j] = j + charge[j] - 1   (the ELU "-1" of layer 3 folded in)
    table2 = np.arange(charge.shape[0], dtype=np.float32) + charge - 1.0
    add = table2[part]

    x_pad = np.zeros((ntot, D), dtype=BF16)
    x_pad[:n] = x.astype(BF16)
    add_pad = np.zeros(ntot, dtype=np.float32)
    add_pad[:n] = add

    # weights: block-diag lhsT, biases with the previous layer's "-1" folded
    lhsT = [np.kron(np.eye(G, dtype=np.float32), W.T).astype(BF16) for W in (W1, W2, W3)]
    b2p = b2 - W2.sum(axis=1)
    b3p = b3 - W3.sum(axis=1)
    biases = [np.tile(b, G).reshape(128, 1).astype(np.float32) for b in (b1, b2p, b3p)]
    ident = np.eye(128, dtype=np.float32).astype(BF16)

    in_maps = []
    for c in range(NCORES):
        xs = x_pad[c * npc : (c + 1) * npc]
        ad = add_pad[c * npc : (c + 1) * npc]
        # addT[t, b*32 + k*8 + g] = add[b*4096 + k*1024 + t*8 + g]
        addT = (
            ad.reshape(-1, 4, 128, G).transpose(2, 0, 1, 3).reshape(128, -1)
        ).copy()
        in_maps.append(
            {
                "x": xs,
                "addT": addT,
                "lhsT1": lhsT[0], "lhsT2": lhsT[1], "lhsT3": lhsT[2],
                "bias1": biases[0], "bias2": biases[1], "bias3": biases[2],
                "ident": ident,
            }
        )
    return in_maps, n, npc


def kernel(**inputs) -> np.ndarray:
    from concourse.bass_utils import run_bass_kernel_spmd

    in_maps, n, npc = _host_prep(**inputs)
    nc = _build_program(npc // MEGA_ATOMS)
    res = run_bass_kernel_spmd(nc, in_maps, core_ids=list(range(NCORES)))
    out = np.concatenate([res.results[c]["out"] for c in range(NCORES)], axis=0)
    return out[:n].astype(np.float32)


# ------------------------- numpy reference of the device math -------------------
def numpy_ref(x, charge, W1, b1, W2, b2, W3, b3, part):
    def elu(v):
        return np.where(v > 0, v, np.expm1(v))

    h = elu(x @ W1.T + b1)
    h = elu(h @ W2.T + b2)
    h = elu(h @ W3.T + b3)
    add = part.astype(np.float32) + charge[part]
    return h + add[:, None]


# revision 8
# speedup vs baseline: 40.5116x; 1.6015x over previous
"""Trainium2 Bass kernel for nn_Nodes_Embedding (dense MLP + per-atom scalar add).

out[i] = ELU3(x[i]) + atom_part[i] + atom_charge[atom_part[i]]

Strategy (pure data parallel over 8 cores):
  - Host: gather add[i] = atom_part[i] + atom_charge[atom_part[i]] - 1 (tiny 1024-entry
    table, O(N) numpy), cast x to bf16, shard atoms across 8 cores.
  - Device (per core, feature-major layout):
      * x loaded via DMA-xbar-transpose into [128, C] bf16 tiles: partition 16g+f holds
        feature f of interleaved atom group g (atom = base + 8*t + g).
      * 3 MLP layers as block-diagonal [128,128] bf16 matmuls (8 copies of Wl.T).
      * ELU(x) = max(x,0) + min(exp(x),1) - 1:
          ACT: t = Exp(psum + b)   (free per-partition bias)
          DVE: r = max(psum + b, 0)  (dual-op tensor_scalar)
          DVE/GPSIMD: m = min(t, 1)
        r and m fed as two accumulated matmuls into the next layer ("-1" folded into
        the next layer's bias: b' = b - W @ ones).
      * transpose back via PE (bf16), final DVE pass adds the fp32 per-atom scalar
        (broadcast along features) and writes fp32 output tiles, contiguous DMA out.
"""

import os
import sys
import functools

import numpy as np

sys.path.insert(0, "/opt/trn_rl_repo")

import ml_dtypes

BF16 = ml_dtypes.bfloat16

D = 16
G = 8                       # partition groups of 16 features
MEGA_COLS = 2048            # columns per load megatile (per group)
MEGA_ATOMS = G * MEGA_COLS  # 16384 atoms per megatile
CHUNK_COLS = 1024           # compute chunk (2-bank PSUM)
MM_COLS = 512               # single matmul free dim
NCORES = 8

N_ATOMS = 4_000_000
N_PARTS = 1024


def _round_up(a, b):
    return (a + b - 1) // b * b


@functools.lru_cache(maxsize=4)
def _build_program(megas, min_on_gpsimd=False):
    """Build the per-core Bass program for `megas` megatiles of atoms."""
    import concourse.bass as bass
    import concourse.bacc as bacc
    import concourse.tile as tile
    from concourse import mybir
    from contextlib import ExitStack

    dt = mybir.dt
    AF = mybir.ActivationFunctionType
    OP = mybir.AluOpType

    npc = megas * MEGA_ATOMS

    nc = bacc.Bacc("TRN2", target_bir_lowering=False, debug=False)

    x_d = nc.dram_tensor("x", [npc, D], dt.bfloat16, kind="ExternalInput").ap()
    addT_d = nc.dram_tensor(
        "addT", [128, npc // 128], dt.float32, kind="ExternalInput"
    ).ap()
    lt_d = [
        nc.dram_tensor(f"lhsT{l}", [128, 128], dt.bfloat16, kind="ExternalInput").ap()
        for l in (1, 2, 3)
    ]
    bias_d = [
        nc.dram_tensor(f"bias{l}", [128, 1], dt.float32, kind="ExternalInput").ap()
        for l in (1, 2, 3)
    ]
    ident_d = nc.dram_tensor(
        "ident", [128, 128], dt.bfloat16, kind="ExternalInput"
    ).ap()
    out_d = nc.dram_tensor("out", [npc, D], dt.float32, kind="ExternalOutput").ap()

    # atom = m*MEGA_ATOMS + 8*t + e  (interleaved groups)
    # One contiguous xbar transpose per megatile: source rows of 128 elements
    # (= 8 atoms x 16 feats), so dest partition 16g+f = feat f of atom 8t+g.
    x2 = x_d.rearrange("(m t e) d -> m t (e d)", e=G, t=MEGA_COLS)
    # output blocks of 1024 atoms: atom = q*1024 + 8*t + e
    out5 = out_d.rearrange("(q t e) d -> t q (e d)", t=128, e=G)

    with ExitStack() as ctx:
        tc = ctx.enter_context(tile.TileContext(nc))
        consts = ctx.enter_context(tc.tile_pool(name="consts", bufs=1))
        xpool = ctx.enter_context(tc.tile_pool(name="xpool", bufs=2))
        actp = ctx.enter_context(tc.tile_pool(name="actp", bufs=3))
        addp = ctx.enter_context(tc.tile_pool(name="addp", bufs=2))
        outp = ctx.enter_context(tc.tile_pool(name="outp", bufs=3))
        psums = ctx.enter_context(tc.tile_pool(name="psums", bufs=3, space="PSUM"))
        psumT = ctx.enter_context(tc.tile_pool(name="psumT", bufs=2, space="PSUM"))

        LT = []
        for l in range(3):
            t = consts.tile([128, 128], dt.bfloat16, tag=f"lt{l}")
            nc.sync.dma_start(t, lt_d[l])
            LT.append(t)
        BIAS = []
        for l in range(3):
            t = consts.tile([128, 1], dt.float32, tag=f"b{l}")
            nc.sync.dma_start(t, bias_d[l])
            BIAS.append(t)
        IDENT = consts.tile([128, 128], dt.bfloat16, tag="ident")
        nc.sync.dma_start(IDENT, ident_d)

        n_chunks = MEGA_COLS // CHUNK_COLS
        halves = CHUNK_COLS // MM_COLS

        for m in range(megas):
            X = xpool.tile([128, MEGA_COLS], dt.bfloat16, tag="X")
            nc.sync.dma_start_transpose(X, x2[m])
            # add values for this megatile: [128, 4*8] per 4096-atom block, 4 blocks
            AT = addp.tile([128, MEGA_ATOMS // 128], dt.float32, tag="AT")
            nc.sync.dma_start(
                AT, addT_d[:, m * (MEGA_ATOMS // 128) : (m + 1) * (MEGA_ATOMS // 128)]
            )

            # layer-major over the chunks of this megatile: one LDWEIGHTS per layer
            ps = [None] * n_chunks
            T = [None] * n_chunks
            R = [None] * n_chunks
            M = [None] * n_chunks
            Cc = [None] * n_chunks

            # ---- layer 1 matmuls (weights LT[0] stationary across all)
            for c in range(n_chunks):
                Xc = X[:, c * CHUNK_COLS : (c + 1) * CHUNK_COLS]
                ps[c] = psums.tile([128, CHUNK_COLS], dt.float32, tag="ps", name=f"ps1_{m}_{c}")
                for h in range(halves):
                    s = slice(h * MM_COLS, (h + 1) * MM_COLS)
                    nc.tensor.matmul(ps[c][:, s], LT[0], Xc[:, s], start=True, stop=True)
            for c in range(n_chunks):
                T[c] = actp.tile([128, CHUNK_COLS], dt.bfloat16, tag="T", name=f"T_{m}_{c}")
                nc.scalar.activation(T[c], ps[c], AF.Exp, bias=BIAS[0])
                R[c] = actp.tile([128, CHUNK_COLS], dt.bfloat16, tag="R", name=f"R_{m}_{c}")
                nc.vector.tensor_scalar(
                    out=R[c], in0=ps[c], scalar1=BIAS[0], scalar2=0.0,
                    op0=OP.add, op1=OP.max,
                )
                M[c] = actp.tile([128, CHUNK_COLS], dt.bfloat16, tag="M", name=f"M_{m}_{c}")
                nc.vector.tensor_scalar(
                    out=M[c], in0=T[c], scalar1=1.0, scalar2=None, op0=OP.min
                )

            # ---- layer 2
            for c in range(n_chunks):
                p2 = psums.tile([128, CHUNK_COLS], dt.float32, tag="ps")
                for h in range(halves):
                    s = slice(h * MM_COLS, (h + 1) * MM_COLS)
                    nc.tensor.matmul(p2[:, s], LT[1], R[c][:, s], start=True, stop=False)
                    nc.tensor.matmul(p2[:, s], LT[1], M[c][:, s], start=False, stop=True)
                ps[c] = p2
            for c in range(n_chunks):
                T[c] = actp.tile([128, CHUNK_COLS], dt.bfloat16, tag="T", name=f"T_{m}_{c}")
                nc.scalar.activation(T[c], ps[c], AF.Exp, bias=BIAS[1])
                R[c] = actp.tile([128, CHUNK_COLS], dt.bfloat16, tag="R", name=f"R_{m}_{c}")
                nc.vector.tensor_scalar(
                    out=R[c], in0=ps[c], scalar1=BIAS[1], scalar2=0.0,
                    op0=OP.add, op1=OP.max,
                )
                M[c] = actp.tile([128, CHUNK_COLS], dt.bfloat16, tag="M", name=f"M_{m}_{c}")
                nc.vector.tensor_scalar(
                    out=M[c], in0=T[c], scalar1=1.0, scalar2=None, op0=OP.min
                )

            # ---- layer 3 (relu on ACT to balance engines)
            for c in range(n_chunks):
                p3 = psums.tile([128, CHUNK_COLS], dt.float32, tag="ps")
                for h in range(halves):
                    s = slice(h * MM_COLS, (h + 1) * MM_COLS)
                    nc.tensor.matmul(p3[:, s], LT[2], R[c][:, s], start=True, stop=False)
                    nc.tensor.matmul(p3[:, s], LT[2], M[c][:, s], start=False, stop=True)
                ps[c] = p3
            for c in range(n_chunks):
                T[c] = actp.tile([128, CHUNK_COLS], dt.bfloat16, tag="T", name=f"T_{m}_{c}")
                nc.scalar.activation(T[c], ps[c], AF.Exp, bias=BIAS[2])
                R[c] = actp.tile([128, CHUNK_COLS], dt.bfloat16, tag="R", name=f"R_{m}_{c}")
                nc.scalar.activation(R[c], ps[c], AF.Relu, bias=BIAS[2])
                M[c] = actp.tile([128, CHUNK_COLS], dt.bfloat16, tag="M", name=f"M_{m}_{c}")
                nc.vector.tensor_scalar(
                    out=M[c], in0=T[c], scalar1=1.0, scalar2=None, op0=OP.min
                )
                Cc[c] = actp.tile([128, CHUNK_COLS], dt.bfloat16, tag="C", name=f"C_{m}_{c}")
                nc.vector.tensor_tensor(out=Cc[c], in0=R[c], in1=M[c], op=OP.add)

            # ---- transpose back + final add, per 512 cols (4096 atoms)
            for c in range(n_chunks):
                for half in range(halves):
                    pT = psumT.tile([128, MM_COLS], dt.bfloat16, tag="pT")
                    for k in range(4):
                        nc.tensor.transpose(
                            pT[:, k * 128 : (k + 1) * 128],
                            Cc[c][:, half * MM_COLS + k * 128 : half * MM_COLS + (k + 1) * 128],
                            IDENT,
                        )
                    osb = outp.tile([128, MM_COLS], dt.float32, tag="osb")
                    # add slice for this 4096-atom span: [128, 4 blocks, 8 groups]
                    a0 = (c * halves + half) * 32
                    sl = AT[:, a0 : a0 + 32]
                    add_b = bass.AP(
                        tensor=sl.tensor,
                        offset=sl.offset,
                        ap=[sl.ap[0], [8, 4], [1, 8], [0, 16]],
                    )
                    nc.vector.tensor_tensor(
                        out=osb.rearrange("p (k g f) -> p k g f", k=4, g=8),
                        in0=pT.rearrange("p (k g f) -> p k g f", k=4, g=8),
                        in1=add_b,
                        op=OP.add,
                    )
                    q0 = m * (MEGA_ATOMS // 1024) + (c * halves + half) * 4
                    nc.sync.dma_start(
                        out5[:, q0 : q0 + 4, :],
                        osb.rearrange("p (k f) -> p k f", k=4),
                    )

    nc.compile()
    return nc


def _host_prep(atom_type_vector, atom_charge, W1, b1, W2, b2, W3, b3, atom_part):
    """Host-side: gather table, pad, shard, reorder. Returns (in_maps, n, npc)."""
    x = np.asarray(atom_type_vector, dtype=np.float32)
    charge = np.asarray(atom_charge, dtype=np.float32)
    part = np.asarray(atom_part).astype(np.int64)
    W1 = np.asarray(W1, np.float32); b1 = np.asarray(b1, np.float32)
    W2 = np.asarray(W2, np.float32); b2 = np.asarray(b2, np.float32)
    W3 = np.asarray(W3, np.float32); b3 = np.asarray(b3, np.float32)

    n = x.shape[0]
    npc = _round_up(_round_up(n, NCORES) // NCORES, MEGA_ATOMS)
    ntot = npc * NCORES

    # table2[j] = j + charge[j] - 1   (the ELU "-1" of layer 3 folded in)
    table2 = np.arange(charge.shape[0], dtype=np.float32) + charge - 1.0
    add = table2[part]

    x_pad = np.zeros((ntot, D), dtype=BF16)
    x_pad[:n] = x.astype(BF16)
    add_pad = np.zeros(ntot, dtype=np.float32)
    add_pad[:n] = add

    # weights: block-diag lhsT, biases with the previous layer's "-1" folded
    lhsT = [np.kron(np.eye(G, dtype=np.float32), W.T).astype(BF16) for W in (W1, W2, W3)]
    b2p = b2 - W2.sum(axis=1)
    b3p = b3 - W3.sum(axis=1)
    biases = [np.tile(b, G).reshape(128, 1).astype(np.float32) for b in (b1, b2p, b3p)]
    ident = np.eye(128, dtype=np.float32).astype(BF16)

    in_maps = []
    for c in range(NCORES):
        xs = x_pad[c * npc : (c + 1) * npc]
        ad = add_pad[c * npc : (c + 1) * npc]
        # addT[t, b*32 + k*8 + g] = add[b*4096 + k*1024 + t*8 + g]
        addT = (
            ad.reshape(-1, 4, 128, G).transpose(2, 0, 1, 3).reshape(128, -1)
        ).copy()
        in_maps.append(
            {
                "x": xs,
                "addT": addT,
                "lhsT1": lhsT[0], "lhsT2": lhsT[1], "lhsT3": lhsT[2],
                "bias1": biases[0], "bias2": biases[1], "bias3": biases[2],
                "ident": ident,
            }
        )
    return in_maps, n, npc


def kernel(**inputs) -> np.ndarray:
    from concourse.bass_utils import run_bass_kernel_spmd

    in_maps, n, npc = _host_prep(**inputs)
    nc = _build_program(npc // MEGA_ATOMS)
    res = run_bass_kernel_spmd(nc, in_maps, core_ids=list(range(NCORES)))
    out = np.concatenate([res.results[c]["out"] for c in range(NCORES)], axis=0)
    return out[:n].astype(np.float32)


# ------------------------- numpy reference of the device math -------------------
def numpy_ref(x, charge, W1, b1, W2, b2, W3, b3, part):
    def elu(v):
        return np.where(v > 0, v, np.expm1(v))

    h = elu(x @ W1.T + b1)
    h = elu(h @ W2.T + b2)
    h = elu(h @ W3.T + b3)
    add = part.astype(np.float32) + charge[part]
    return h + add[:, None]
